# revision 4
# baseline (speedup 1.0000x reference)
"""Trainium2 Bass kernel for the 8-level butterfly layer (v2).

Contract: kernel(**inputs) takes FULL unsharded numpy inputs
(in_data [512,4096], W_in [16,64], b_in [64], W_lvl [510,2,64,64],
b_lvl [510,64], Fea [256,64,16]) and returns the FULL output
(512, 4096, 1) float32.

Sharding: 2 subtree halves x 4 batch quarters. Core c = sub*4 + bq
handles batch rows [bq*128,(bq+1)*128) and level-1 subtree `sub`
(boxes [sub*2^(L-1), ...) at level L). Input stage is computed for all
256 k-blocks (both level-1 children need every level-0 position);
levels 1..8 and the output touch only the core's subtree half, so the
per-level box count (and thus weight-load traffic) is half that of pure
batch sharding, with 128-batch columns doubling every matmul stream.

Activation layout per level L>=1 ("pair format"), R_L [128, 8192] bf16:
box b (core-local) occupies columns [b*N, (b+1)*N), N = 8192/2^(L-1);
partition (s*64 + ch) holds channel ch of position t with parity s;
column within the box block is (t//2)*128 + batch. R_0 is [128, 16384]
(128 k-pairs x 128 batch). R_8 is pair-stacked: partition (beta*64+ch),
column (pair*128 + batch).

Biases: levels in..5 are per-partition scalars applied in the evac
(relu(x+b) on ACT / add+max on DVE). Levels 6-8 fold biases into
per-level shifts c_L (host-side recurrence); the evac computes
max(psum, -c_L) with a column-broadcast negc operand on the DVE, so no
PE cycles are spent on bias matmuls. The output stage's Fea^T c_8
constant is injected with a k=1 ones-row matmul into PSUM before the
pair matmuls accumulate.
"""

import numpy as np
import ml_dtypes

import concourse.bass as bass
import concourse.mybir as mybir
import concourse.tile as tile
from concourse import bacc

NCORES = 8
NSUB = 2
NBQ = 4
B = 512
BC = B // NBQ  # 128 batch rows per core
NLVL = 8
C = 64
FIN = 16
FOUT = 16
INS = 4096

BF16 = mybir.dt.bfloat16
F32 = mybir.dt.float32

_CACHE: dict = {}
_PHASES: list = []

# wall column offsets: level L boxes start at box index 2^(L-1)-1
_WST = {lv: 2 ** (lv - 1) - 1 for lv in range(1, 8)}
NWALL = 127  # local boxes levels 1..7


def _bf16(a: np.ndarray) -> np.ndarray:
    return np.ascontiguousarray(np.asarray(a, np.float32)).astype(ml_dtypes.bfloat16)


def _dupT(b: np.ndarray) -> np.ndarray:
    """[nb, 64] -> [128, nb] with both partition halves holding b."""
    return np.ascontiguousarray(np.concatenate([b, b], axis=1).T, np.float32)


def pack_shared(W_in, b_in, W_lvl, b_lvl, Fea, sub: int) -> dict:
    """Host-side packing of the filter tensors for subtree half `sub`."""
    W_in = np.asarray(W_in, np.float32)
    b_in = np.asarray(b_in, np.float32)
    W_lvl = np.asarray(W_lvl, np.float32)
    b_lvl = np.asarray(b_lvl, np.float32)
    Fea = np.asarray(Fea, np.float32)

    def lslice(lv):
        nb = 2 ** (lv - 1)
        g0 = 2 ** lv - 2 + sub * nb
        return W_lvl[g0:g0 + nb], b_lvl[g0:g0 + nb], nb

    # levels 1..7 weights -> wall [128, 127*64], row = s*64 + c_in
    wall = np.zeros((128, NWALL * 64), np.float32)
    bd_cols = []
    for lv in range(1, 8):
        W, bb, nb = lslice(lv)
        blk = W.reshape(nb, 128, 64).transpose(1, 0, 2).reshape(128, nb * 64)
        st = _WST[lv] * 64
        wall[:, st:st + nb * 64] = blk
        if lv <= 5:
            bd_cols.append(_dupT(bb))
    bd = np.concatenate(bd_cols, axis=1)  # [128, 31]

    # shift recurrence for levels 6..8 (bf16-rounded weights for
    # consistency with the device data path)
    def wbf(lv):
        W, bb, nb = lslice(lv)
        return _bf16(W).astype(np.float32), bb, nb

    W6, b6, _ = wbf(6)
    c6 = b6  # beta_5 = 0
    W7, b7, _ = wbf(7)
    c7 = b7 + np.einsum("kscd,kc->kd", W7, np.repeat(c6, 2, axis=0))
    W8, b8, _ = wbf(8)
    c8 = b8 + np.einsum("kscd,kc->kd", W8, np.repeat(c7, 2, axis=0))

    negc6 = -_dupT(c6)  # [128, 32]
    negc7 = -_dupT(c7)  # [128, 64]
    # l8 pair-stacked: [128 (beta*64+d), 64 pairs]
    negc8 = np.ascontiguousarray(
        -c8.reshape(64, 2, 64).transpose(1, 2, 0).reshape(128, 64), np.float32)

    # level 8 weights pair-stacked: [128 (s*64+c), 64*128 (p, beta*64+d)]
    w8 = W8.reshape(64, 2, 2, C, C).transpose(2, 3, 0, 1, 4).reshape(128, 64 * 128)

    # Fea blockdiag pairs: [128 (beta*64+c), 64*32 (p, beta*16+f)]
    Feas = Fea[sub * 128:(sub + 1) * 128]
    fea = np.zeros((128, 64, 32), np.float32)
    fea[0:64, :, 0:16] = Feas[0::2].transpose(1, 0, 2)
    fea[64:128, :, 16:32] = Feas[1::2].transpose(1, 0, 2)
    fea = fea.reshape(128, 64 * 32)

    # out-stage constant Fea^T c8, K=128-padded: row 0 = (p, beta*16+f)
    oc = np.einsum("kcf,kc->kf", _bf16(Feas).astype(np.float32), c8)
    outconst = np.zeros((128, 2048), np.float32)
    outconst[0, :] = oc.reshape(64, 2, 16).reshape(2048)
    onesP = np.zeros((128, 128), np.float32)
    onesP[0, :] = 1.0

    # input filter, k-parity blockdiag, replicated for 4 row tiles
    w1 = np.zeros((32, 128), np.float32)
    w1[0:16, 0:64] = W_in
    w1[16:32, 64:128] = W_in
    winp = np.tile(w1, (4, 1))
    bin_h = np.concatenate([b_in, b_in]).reshape(128, 1)

    return {
        "winp": _bf16(winp),
        "wall": _bf16(wall),
        "w8": _bf16(w8),
        "fea": _bf16(fea),
        "bin": np.ascontiguousarray(bin_h, np.float32),
        "bd": np.ascontiguousarray(bd, np.float32),
        "negc6": negc6,
        "negc7": negc7,
        "negc8": negc8,
        "outconst": _bf16(outconst),
        "ones1": _bf16(onesP),
    }


def pack_x(x_shard: np.ndarray) -> np.ndarray:
    """[128, 4096] batch shard (full k range) -> [128, 4096] bf16 for the
    4x row-tiled input stage: row r*32+h*16+f holds x[b, (2k'+h)*16+f]
    at col (k'-32r)*128+b, where r = k'//32."""
    xs = np.asarray(x_shard, np.float32).reshape(BC, 4, 32, 2, FIN)
    # [b, r, j, h, f] -> [r, h, f, j, b]
    return _bf16(xs.transpose(1, 3, 4, 2, 0).reshape(128, 32 * BC))


def make_in_maps(inputs) -> list[dict]:
    in_data = np.asarray(inputs["in_data"], np.float32)
    shared = [
        pack_shared(inputs["W_in"], inputs["b_in"], inputs["W_lvl"],
                    inputs["b_lvl"], inputs["Fea"], sub)
        for sub in range(NSUB)
    ]
    in_maps = []
    for c in range(NCORES):
        sub, bq = c // NBQ, c % NBQ
        m = dict(shared[sub])
        m["xt"] = pack_x(in_data[bq * BC:(bq + 1) * BC])
        in_maps.append(m)
    return in_maps


def unpack_out(out_stacked: np.ndarray) -> np.ndarray:
    """[8*128, 2048] stacked core outputs -> [512, 4096, 1] float32."""
    full = np.zeros((B, INS), np.float32)
    for c in range(NCORES):
        sub, bq = c // NBQ, c % NBQ
        full[bq * BC:(bq + 1) * BC, sub * 2048:(sub + 1) * 2048] = \
            out_stacked[c * BC:(c + 1) * BC]
    return full.reshape(B, INS, 1)


def _build_module(loop_iters: int | None = None):
    nc = bacc.Bacc("TRN2", target_bir_lowering=False, debug=False)

    xt = nc.dram_tensor("xt", [128, 4096], BF16, kind="ExternalInput")
    winp = nc.dram_tensor("winp", [128, 128], BF16, kind="ExternalInput")
    wall = nc.dram_tensor("wall", [128, NWALL * 64], BF16, kind="ExternalInput")
    w8 = nc.dram_tensor("w8", [128, 64 * 128], BF16, kind="ExternalInput")
    fea = nc.dram_tensor("fea", [128, 64 * 32], BF16, kind="ExternalInput")
    bin_t = nc.dram_tensor("bin", [128, 1], F32, kind="ExternalInput")
    bd_t = nc.dram_tensor("bd", [128, 31], F32, kind="ExternalInput")
    negc6_t = nc.dram_tensor("negc6", [128, 32], F32, kind="ExternalInput")
    negc7_t = nc.dram_tensor("negc7", [128, 64], F32, kind="ExternalInput")
    negc8_t = nc.dram_tensor("negc8", [128, 64], F32, kind="ExternalInput")
    outconst_t = nc.dram_tensor("outconst", [128, 2048], BF16, kind="ExternalInput")
    ones1_t = nc.dram_tensor("ones1", [128, 128], BF16, kind="ExternalInput")
    out = nc.dram_tensor("out", [BC, 2048], F32, kind="ExternalOutput")

    relu = mybir.ActivationFunctionType.Relu
    add = mybir.AluOpType.add
    amax = mybir.AluOpType.max

    _PHASES.clear()

    def mark(name):
        _PHASES.append((name, int(nc.get_next_instruction_name().split("-")[1])))

    # evac engine balance: running cost estimate per engine (ns).
    # DVE is pre-charged with the level-6..8 shift evacs (48 ops, DVE-pinned)
    # so the earlier flexible evacs favor ACT in anticipation.
    ebal = {"act": 0.0, "dve": 48 * 700.0}

    def evac_flex(dst, src, bias_ap, cols):
        if ebal["act"] <= ebal["dve"]:
            nc.scalar.activation(dst, src, relu,
                                 bias=bias_ap if bias_ap is not None else 0.0)
            ebal["act"] += 174 + cols
        elif bias_ap is not None:
            nc.vector.tensor_scalar(dst, src, bias_ap, 0.0, op0=add, op1=amax)
            ebal["dve"] += 178 + cols
        else:
            nc.vector.tensor_scalar(dst, src, 0.0, None, op0=amax)
            ebal["dve"] += 178 + cols

    def evac_shift(dst, src, negc_ap, ngrp, cols):
        """dst = max(src, negc) with negc [128, ngrp] broadcast over
        cols//ngrp columns per group. DVE-pinned."""
        per = cols // ngrp
        nc.vector.scalar_tensor_tensor(
            dst.rearrange("a (n z) -> a n z", n=ngrp),
            src.rearrange("a (n z) -> a n z", n=ngrp),
            0.0, negc_ap.broadcast_to([128, ngrp, per]),
            op0=add, op1=amax)
        ebal["dve"] += 240 + cols

    import contextlib

    with tile.TileContext(nc) as tc:
        with (
            tc.tile_pool(name="wp", bufs=1) as wp,
            tc.tile_pool(name="xp", bufs=1) as xp,
            tc.tile_pool(name="r0p", bufs=1) as r0p,
            tc.tile_pool(name="rp", bufs=3) as rp,
            tc.tile_pool(name="op", bufs=3) as op,
            tc.tile_pool(name="ps", bufs=6, space="PSUM") as ps,
            tc.tile_pool(name="po", bufs=2, space="PSUM") as po,
            tc.For_i(0, loop_iters, 1) if loop_iters else contextlib.nullcontext(),
        ):
            # ---- DMAs (in rough use order; Tile overlaps them).
            # x is split across the sync and gpsimd DMA queues.
            x_sb = xp.tile([128, 4096], BF16, tag="x")
            for h in range(4):
                eng = nc.sync if h % 2 == 0 else nc.gpsimd
                eng.dma_start(x_sb[:, h * 1024:(h + 1) * 1024],
                              xt.ap()[:, h * 1024:(h + 1) * 1024])
            winp_sb = wp.tile([128, 128], BF16, tag="winp")
            nc.sync.dma_start(winp_sb[:, :], winp.ap())
            bin_sb = wp.tile([128, 1], F32, tag="bin")
            nc.sync.dma_start(bin_sb[:, :], bin_t.ap())
            bd_sb = wp.tile([128, 31], F32, tag="bd")
            nc.sync.dma_start(bd_sb[:, :], bd_t.ap())
            wall_sb = wp.tile([128, NWALL * 64], BF16, tag="wall")
            # split: lv1-5 boxes (31*64 cols), lv6 (32*64), lv7 (64*64)
            nc.sync.dma_start(wall_sb[:, 0:31 * 64], wall.ap()[:, 0:31 * 64])
            nc.gpsimd.dma_start(wall_sb[:, 31 * 64:63 * 64],
                                wall.ap()[:, 31 * 64:63 * 64])
            nc.gpsimd.dma_start(wall_sb[:, 63 * 64:127 * 64],
                                wall.ap()[:, 63 * 64:127 * 64])
            negc_sb = {}
            for nm, t, w in (("negc6", negc6_t, 32), ("negc7", negc7_t, 64),
                             ("negc8", negc8_t, 64)):
                negc_sb[nm] = wp.tile([128, w], F32, tag=nm, name=nm)
                nc.sync.dma_start(negc_sb[nm][:, :], t.ap())
            w8_sb = wp.tile([128, 64 * 128], BF16, tag="w8")
            nc.sync.dma_start(w8_sb[:, 0:4096], w8.ap()[:, 0:4096])
            nc.gpsimd.dma_start(w8_sb[:, 4096:8192], w8.ap()[:, 4096:8192])
            fea_sb = wp.tile([128, 64 * 32], BF16, tag="fea")
            nc.sync.dma_start(fea_sb[:, :], fea.ap())
            outconst_sb = wp.tile([128, 2048], BF16, tag="outconst")
            nc.sync.dma_start(outconst_sb[:, :], outconst_t.ap())
            ones1_sb = wp.tile([128, 128], BF16, tag="ones1")
            nc.sync.dma_start(ones1_sb[:, :], ones1_t.ap())

            def wt(lv, box):
                st = (_WST[lv] + box) * 64
                return wall_sb[:, st:st + 64]

            # ---- input stage: 4x row-tiled (32-deep blockdiag filters on
            # four concurrent 32x128 PE tiles), 32 chunks of 512 -> R0
            mark("input")
            R0 = r0p.tile([128, 16384], BF16, tag="R0")
            for j in range(8):
                for r in range(4):
                    pc = ps.tile([128, 512], F32, tag="ps")
                    nc.tensor.matmul(
                        pc[:, :], lhsT=winp_sb[32 * r:32 * (r + 1), :],
                        rhs=x_sb[32 * r:32 * (r + 1), j * 512:(j + 1) * 512],
                        start=True, stop=True, tile_position=(32 * r, 0),
                    )
                    evac_flex(R0[:, r * 4096 + j * 512:r * 4096 + (j + 1) * 512],
                              pc[:, :], bin_sb[:, :], 512)

            # ---- level 1: single box; parent = R0 (16384 cols)
            mark("l1")
            R = rp.tile([128, 8192], BF16, tag="R")
            pv = R0[:, :].rearrange("a (t2 two b) -> a t2 two b", two=2, b=BC)
            for j in range(16):
                pc = ps.tile([128, 512], F32, tag="ps")
                for q in range(2):
                    nc.tensor.matmul(
                        pc[64 * q:64 * (q + 1), :], lhsT=wt(1, 0),
                        rhs=pv[:, j * 4:(j + 1) * 4, q, :],
                        start=True, stop=True, tile_position=(0, 64 * q),
                    )
                evac_flex(R[:, j * 512:(j + 1) * 512], pc[:, :],
                          bd_sb[:, 0:1], 512)

            # ---- levels 2..5: per-box chunks, V-form evac with bias
            for lv in range(2, 6):
                mark(f"l{lv}")
                nb = 2 ** (lv - 1)           # children this level
                Np = 8192 // (nb // 2)       # parent block columns
                Ncb = Np // 2                # child block columns
                nch = Ncb // 512             # chunks per child
                Rn = rp.tile([128, 8192], BF16, tag="R")
                for p in range(nb // 2):
                    pv = R[:, p * Np:(p + 1) * Np].rearrange(
                        "a (t2 two b) -> a t2 two b", two=2, b=BC)
                    for cl in range(2):
                        box = 2 * p + cl
                        lhsT = wt(lv, box)
                        for j in range(nch):
                            pc = ps.tile([128, 512], F32, tag="ps")
                            for q in range(2):
                                nc.tensor.matmul(
                                    pc[64 * q:64 * (q + 1), :], lhsT=lhsT,
                                    rhs=pv[:, j * 4:(j + 1) * 4, q, :],
                                    start=True, stop=True,
                                    tile_position=(0, 64 * q),
                                )
                            bc = _WST[lv] + box
                            evac_flex(
                                Rn[:, box * Ncb + j * 512:box * Ncb + (j + 1) * 512],
                                pc[:, :], bd_sb[:, bc:bc + 1], 512)
                R = Rn

            # ---- level 6: 32 boxes x 256 cols; chunk = 2 siblings
            mark("l6")
            Rn = rp.tile([128, 8192], BF16, tag="R")
            for i in range(16):
                pc = ps.tile([128, 512], F32, tag="ps")
                pv = R[:, i * 512:(i + 1) * 512].rearrange(
                    "a (t2 two b) -> a t2 two b", two=2, b=BC)
                for cl in range(2):
                    box = 2 * i + cl
                    lhsT = wt(6, box)
                    for q in range(2):
                        nc.tensor.matmul(
                            pc[64 * q:64 * (q + 1), cl * 256:(cl + 1) * 256],
                            lhsT=lhsT, rhs=pv[:, :, q, :],
                            start=True, stop=True, tile_position=(0, 64 * q),
                        )
                evac_shift(Rn[:, i * 512:(i + 1) * 512], pc[:, :],
                           negc_sb["negc6"][:, 2 * i:2 * i + 2], 2, 512)
            R = Rn

            # ---- level 7: 64 boxes x 128 cols; chunk = 4 boxes
            mark("l7")
            Rn = rp.tile([128, 8192], BF16, tag="R")
            for i in range(16):
                pc = ps.tile([128, 512], F32, tag="ps")
                for half in range(2):
                    p = 2 * i + half
                    pv = R[:, p * 256:(p + 1) * 256].rearrange(
                        "a (t2 two b) -> a t2 two b", two=2, b=BC)
                    for cl in range(2):
                        box = 2 * p + cl
                        lhsT = wt(7, box)
                        for q in range(2):
                            nc.tensor.matmul(
                                pc[64 * q:64 * (q + 1),
                                   (2 * half + cl) * 128:(2 * half + cl + 1) * 128],
                                lhsT=lhsT, rhs=pv[:, 0:1, q, :],
                                start=True, stop=True,
                                tile_position=(0, 64 * q),
                            )
                evac_shift(Rn[:, i * 512:(i + 1) * 512], pc[:, :],
                           negc_sb["negc7"][:, 4 * i:4 * i + 4], 4, 512)
            R = Rn

            # ---- level 8 (pair-stacked, full-width matmuls) + out stage
            mark("l8")
            Rn = rp.tile([128, 8192], BF16, tag="R")
            for i in range(16):
                pc = ps.tile([128, 512], F32, tag="ps")
                for pl in range(4):
                    p = 4 * i + pl
                    nc.tensor.matmul(
                        pc[:, pl * 128:(pl + 1) * 128],
                        lhsT=w8_sb[:, p * 128:(p + 1) * 128],
                        rhs=R[:, p * 128:(p + 1) * 128],
                        start=True, stop=True,
                    )
                evac_shift(Rn[:, i * 512:(i + 1) * 512], pc[:, :],
                           negc_sb["negc8"][:, 4 * i:4 * i + 4], 4, 512)
                if i % 4 == 3:
                    # out bank t: pairs 16t..16t+15 = l8 chunks 4t..4t+3
                    t = i // 4
                    pco = po.tile([BC, 512], F32, tag="po")
                    nc.tensor.matmul(
                        pco[:, :], lhsT=ones1_sb[:, :],
                        rhs=outconst_sb[:, t * 512:(t + 1) * 512],
                        start=True, stop=False,
                    )
                    for g in range(16):
                        p = 16 * t + g
                        nc.tensor.matmul(
                            pco[:, g * 32:(g + 1) * 32],
                            lhsT=Rn[:, p * 128:(p + 1) * 128],
                            rhs=fea_sb[:, p * 32:(p + 1) * 32],
                            start=False, stop=(g == 15),
                        )
                    o_sb = op.tile([BC, 512], F32, tag="os")
                    if ebal["act"] <= ebal["dve"]:
                        nc.scalar.copy(o_sb[:, :], pco[:, :])
                        ebal["act"] += 174 + 512
                    else:
                        nc.vector.tensor_copy(o_sb[:, :], pco[:, :])
                        ebal["dve"] += 178 + 512
                    nc.sync.dma_start(out.ap()[:, t * 512:(t + 1) * 512],
                                      o_sb[:, :])
            R = Rn

    nc.compile()
    return nc


def _make_runner(nc):
    """Cached jitted SPMD runner over the 8 cores."""
    import jax

    from concourse.bass2jax import (
        _bass_exec_p,
        install_neuronx_cc_hook,
        partition_id_tensor,
    )
    from jax.experimental.shard_map import shard_map
    from jax.sharding import Mesh, PartitionSpec

    install_neuronx_cc_hook()

    partition_name = nc.partition_id_tensor.name if nc.partition_id_tensor else None
    in_names: list[str] = []
    out_names: list[str] = []
    out_avals = []
    zero_outs: list[np.ndarray] = []
    for alloc in nc.m.functions[0].allocations:
        if not isinstance(alloc, mybir.MemoryLocationSet):
            continue
        name = alloc.memorylocations[0].name
        if alloc.kind == "ExternalInput":
            if name != partition_name:
                in_names.append(name)
        elif alloc.kind == "ExternalOutput":
            shape = tuple(alloc.tensor_shape)
            dtype = mybir.dt.np(alloc.dtype)
            out_names.append(name)
            out_avals.append(jax.core.ShapedArray(shape, dtype))
            zero_outs.append(np.zeros(shape, dtype))
    n_params = len(in_names)
    all_names = in_names + out_names
    if partition_name is not None:
        all_names = all_names + [partition_name]

    def _body(*args):
        operands = list(args)
        if partition_name is not None:
            operands.append(partition_id_tensor())
        outs = _bass_exec_p.bind(
            *operands,
            out_avals=tuple(out_avals),
            in_names=tuple(all_names),
            out_names=tuple(out_names),
            lowering_input_output_aliases=(),
            sim_require_finite=True,
            sim_require_nnan=True,
            nc=nc,
        )
        return tuple(outs)

    devices = jax.devices()[:NCORES]
    mesh = Mesh(np.asarray(devices), ("core",))
    n_all = n_params + len(out_names)
    sharded = jax.jit(
        shard_map(
            _body, mesh=mesh,
            in_specs=(PartitionSpec("core"),) * n_all,
            out_specs=(PartitionSpec("core"),) * len(out_names),
            check_rep=False,
        ),
        keep_unused=True,
    )
    return {
        "fn": sharded,
        "in_names": in_names,
        "out_names": out_names,
        "out_avals": out_avals,
        "zero_outs": zero_outs,
    }


def _runner():
    if "nc" not in _CACHE:
        _CACHE["nc"] = _build_module()
    if "runner" not in _CACHE:
        _CACHE["runner"] = _make_runner(_CACHE["nc"])
    return _CACHE["runner"]


def _concat_args(in_maps):
    r = _runner()
    args = [
        np.concatenate([np.asarray(m[name]) for m in in_maps], axis=0)
        for name in r["in_names"]
    ]
    args += [
        np.zeros((NCORES * z.shape[0], *z.shape[1:]), z.dtype) for z in r["zero_outs"]
    ]
    return args


def kernel(**inputs) -> np.ndarray:
    r = _runner()
    in_maps = make_in_maps(inputs)
    out_arrs = r["fn"](*_concat_args(in_maps))
    out = np.asarray(out_arrs[r["out_names"].index("out")])
    return unpack_out(out).astype(np.float32)
